# revision 2
# baseline (speedup 1.0000x reference)
"""KNN WRMF negative sampler on 8 Trainium2 NeuronCores — fused-STT v3.

Data parallel over L=4096: 512 rows/core, row l = t*128 + p (tile t, partition p).
Host precomputes a combined row table indexed by j = loc-1:

    comb[j] = [ p0, nb0, cum[j+1, 0:99], dprob[j+1, 0:99], dknn[j, 0:99] ]  (299 f32)

where p0 = probs[j+1, 0], nb0 = knn[j, 0], dprob = diffs of probs row,
dknn = diffs of knn row (ids exact in f32). One indirect DMA per tile fetches
all per-row data. For each (t, k), one fused DVE op per table:

    acc = sum_n (cum[n] < u) * d[n]   (scalar_tensor_tensor is_lt/mult + accum)
    val = tab[0] + acc                == tab[min(searchsorted_left(cum, u), 99)]

Finalize (val = acc + tab0, f32->i32 for knn) runs on the Activation engine in
two k-chunks per tile so output DMAs overlap the tail of the STT stream.
"""

import numpy as np
from contextlib import ExitStack

import concourse.bass as bass
import concourse.bacc as bacc
import concourse.mybir as mybir
import concourse.tile as tile
from concourse.bass_utils import run_bass_kernel_spmd

P = 128          # partitions
T = 4            # row-tiles per core
RPC = P * T      # rows per core
K = 32           # samples per row
N = 100          # neighbours per row
M = N - 1        # telescoped span
W = 2 + 3 * M    # combined row width (299)
NCORES = 8
NLOC = 100000

_cache = {}


def _build():
    if "nc" in _cache:
        return _cache["nc"]
    nc = bacc.Bacc("TRN2")
    f32, i32 = mybir.dt.float32, mybir.dt.int32
    loc_in = nc.dram_tensor("locm1", [P, T], i32, kind="ExternalInput").ap()
    uni = nc.dram_tensor("uni", [P, T, K], f32, kind="ExternalInput").ap()
    combt = nc.dram_tensor("combt", [NLOC, W], f32, kind="ExternalInput").ap()
    outv = nc.dram_tensor("outv", [P, T, K, 2], f32, kind="ExternalOutput").ap()

    LT = mybir.AluOpType.is_lt
    MULT = mybir.AluOpType.mult
    IDENT = mybir.ActivationFunctionType.Identity

    with tile.TileContext(nc) as tc, ExitStack() as ctx:
        pool = ctx.enter_context(tc.tile_pool(name="m", bufs=1))
        big = ctx.enter_context(tc.tile_pool(name="big", bufs=2))

        loc = pool.tile([P, T], i32)
        nc.sync.dma_start(loc[:], loc_in)
        ut = pool.tile([P, T, K], f32)
        nc.sync.dma_start(ut[:], uni)

        accp = pool.tile([P, T, K], f32)
        accn = pool.tile([P, T, K], f32)
        outt = pool.tile([P, T, K, 2], f32)  # [...,0]=prob, [...,1]=neg id
        NSCR = 8
        scrs = [pool.tile([P, M], f32, name=f"scr{i}", tag=f"scr{i}")
                for i in range(NSCR)]

        HALF = K // 2
        for t in range(T):
            cb = big.tile([P, W], f32, tag="cb")
            nc.gpsimd.indirect_dma_start(
                out=cb[:], out_offset=None, in_=combt[:],
                in_offset=bass.IndirectOffsetOnAxis(ap=loc[:, t:t + 1], axis=0))
            c_ap = cb[:, 2:2 + M]
            dp_ap = cb[:, 2 + M:2 + 2 * M]
            dn_ap = cb[:, 2 + 2 * M:2 + 3 * M]

            for half in range(2):
                for k in range(half * HALF, (half + 1) * HALF):
                    u_s = ut[:, t, k:k + 1]
                    nc.vector.scalar_tensor_tensor(
                        out=scrs[(2 * k) % NSCR][:], in0=c_ap, scalar=u_s,
                        in1=dp_ap, op0=LT, op1=MULT,
                        accum_out=accp[:, t, k:k + 1])
                    nc.vector.scalar_tensor_tensor(
                        out=scrs[(2 * k + 1) % NSCR][:], in0=c_ap, scalar=u_s,
                        in1=dn_ap, op0=LT, op1=MULT,
                        accum_out=accn[:, t, k:k + 1])
                ks = slice(half * HALF, (half + 1) * HALF)
                nc.scalar.activation(
                    out=outt[:, t, ks, 0], in_=accp[:, t, ks], func=IDENT,
                    bias=cb[:, 0:1], scale=1.0)
                nc.scalar.activation(
                    out=outt[:, t, ks, 1], in_=accn[:, t, ks], func=IDENT,
                    bias=cb[:, 1:2], scale=1.0)
                nc.sync.dma_start(
                    outv[:, t, ks, :].rearrange("p k c -> p (k c)"),
                    outt[:, t, ks, :].rearrange("p k c -> p (k c)"))
    nc.compile()
    _cache["nc"] = nc
    return nc


def _prep(trg_seq, uniforms, knn_results, probs_table, cum_probs_table):
    """Host-side shard prep: per-core input dicts + combined table."""
    trg = np.asarray(trg_seq, dtype=np.int32)
    uni = np.ascontiguousarray(np.asarray(uniforms, dtype=np.float32))
    probs = np.asarray(probs_table, dtype=np.float32)
    cum = np.asarray(cum_probs_table, dtype=np.float32)
    knn = np.asarray(knn_results)

    comb = np.empty((NLOC, W), dtype=np.float32)
    comb[:, 0] = probs[1:, 0]
    comb[:, 1] = knn[:, 0]
    comb[:, 2:2 + M] = cum[1:, 0:M]
    comb[:, 2 + M:2 + 2 * M] = probs[1:, 1:] - probs[1:, :-1]
    knnf = knn.astype(np.float32)  # exact: ids < 2^24
    comb[:, 2 + 2 * M:] = knnf[:, 1:] - knnf[:, :-1]
    comb = np.ascontiguousarray(comb)

    in_maps = []
    for c in range(NCORES):
        sl = slice(c * RPC, (c + 1) * RPC)
        locm1 = trg[sl, 1].reshape(T, P).T - 1  # [P, T], row l = t*128+p
        in_maps.append({
            "locm1": np.ascontiguousarray(locm1),
            "uni": np.ascontiguousarray(uni[sl].reshape(T, P, K).transpose(1, 0, 2)),
            "combt": comb,
        })
    return in_maps


def kernel(trg_seq, k, user, uniforms, knn_results, probs_table, cum_probs_table,
           **_ignored):
    nc = _build()
    in_maps = _prep(trg_seq, uniforms, knn_results, probs_table, cum_probs_table)
    res = run_bass_kernel_spmd(nc, in_maps, core_ids=list(range(NCORES)))
    neg = np.empty((NCORES * RPC, K), dtype=np.int32)
    prob = np.empty((NCORES * RPC, K), dtype=np.float32)
    for c in range(NCORES):
        sl = slice(c * RPC, (c + 1) * RPC)
        o = res.results[c]["outv"].transpose(1, 0, 2, 3)  # [T,P,K,2] rows l=t*128+p
        prob[sl] = o[:, :, :, 0].reshape(RPC, K)
        neg[sl] = o[:, :, :, 1].reshape(RPC, K).astype(np.int32)
    return neg, prob


# revision 3
# speedup vs baseline: 1.5409x; 1.5409x over previous
"""KNN WRMF negative sampler on 8 Trainium2 NeuronCores — count+gather v4.

Data parallel over L=4096: 512 rows/core, row l = t*128 + p (tile t, partition p).
Host precomputes a combined row table indexed by j = loc-1:

    comb[j] = [ cum[j+1, 0:99] | interleave(probs[j+1, :], knnf[j, :]) ]  (299 f32)

Per tile:
  1. indirect DMA gathers comb rows (one row per partition).
  2. idx = searchsorted_left(cum, u) clamped to 99 == count of cum[0:99] < u,
     one fused DVE tensor_scalar (is_lt + accumulate) per k at 2x SBUF mode.
  3. gpsimd ap_gather fetches (prob, knn) pairs: its group-wrapped index
     semantics give out[ch, s*16 + ch%16] = in[ch, idx[ch, s]] — the diagonal
     is a true per-partition gather (15/16 of the output is discarded).
  4. The diagonal is extracted with a constant 0/1 mask multiply + reduce on
     DVE, writing final (prob, neg) values; host splits and casts neg to int.

Outputs are exact table entries (no arithmetic on values at all).
"""

import numpy as np
from contextlib import ExitStack

import concourse.bass as bass
import concourse.bacc as bacc
import concourse.mybir as mybir
import concourse.tile as tile
from concourse.bass_utils import run_bass_kernel_spmd

P = 128          # partitions
T = 4            # row-tiles per core
RPC = P * T      # rows per core
K = 32           # samples per row
N = 100          # neighbours per row
M = N - 1        # count span (counting c[0:99] < u builds in the clamp)
W = M + 2 * N    # combined row width (299)
G = 16           # partitions per gpsimd core
NCORES = 8
NLOC = 100000

_cache = {}


def _build():
    if "nc" in _cache:
        return _cache["nc"]
    nc = bacc.Bacc("TRN2")
    f32, i16 = mybir.dt.float32, mybir.dt.int16
    i32 = mybir.dt.int32
    loc_in = nc.dram_tensor("locm1", [P, T], i32, kind="ExternalInput").ap()
    uni = nc.dram_tensor("uni", [P, T, K], f32, kind="ExternalInput").ap()
    combt = nc.dram_tensor("combt", [NLOC, W], f32, kind="ExternalInput").ap()
    mask_in = nc.dram_tensor("mask", [P, G], f32, kind="ExternalInput").ap()
    outv = nc.dram_tensor("outv", [P, T, K, 2], f32, kind="ExternalOutput").ap()

    LT = mybir.AluOpType.is_lt
    ADD = mybir.AluOpType.add
    MULT = mybir.AluOpType.mult
    X = mybir.AxisListType.X

    with tile.TileContext(nc) as tc, ExitStack() as ctx:
        pool = ctx.enter_context(tc.tile_pool(name="m", bufs=1))
        big = ctx.enter_context(tc.tile_pool(name="big", bufs=2))

        loc = pool.tile([P, T], i32)
        nc.sync.dma_start(loc[:], loc_in)
        ut = pool.tile([P, T, K], f32)
        nc.sync.dma_start(ut[:], uni)
        mask = pool.tile([P, G], f32)
        nc.sync.dma_start(mask[:], mask_in)

        outt = pool.tile([P, T, K, 2], f32)  # [...,0]=prob, [...,1]=neg id
        NSCR = 8
        scrs = [pool.tile([P, M], f32, name=f"scr{i}", tag=f"scr{i}")
                for i in range(NSCR)]

        def extract(t, g):
            # diagonal of the group-wrapped gather -> final (prob, neg) values
            gm = big.tile([P, K, G, 2], f32, tag="gm")
            nc.vector.tensor_tensor(
                out=gm[:],
                in0=g[:].rearrange("p (k g) c -> p k g c", g=G),
                in1=mask[:][:, None, :, None].to_broadcast([P, K, G, 2]),
                op=MULT)
            # reduce over G for each c separately (X reduces innermost only)
            nc.vector.tensor_reduce(
                out=outt[:, t, :, 0], in_=gm[:, :, :, 0], axis=X, op=ADD)
            nc.vector.tensor_reduce(
                out=outt[:, t, :, 1], in_=gm[:, :, :, 1], axis=X, op=ADD)
            nc.sync.dma_start(
                outv[:, t, :, :].rearrange("p k c -> p (k c)"),
                outt[:, t, :, :].rearrange("p k c -> p (k c)"))

        gs = [None] * T
        for t in range(T):
            cb = big.tile([P, W], f32, tag="cb")
            nc.gpsimd.indirect_dma_start(
                out=cb[:], out_offset=None, in_=combt[:],
                in_offset=bass.IndirectOffsetOnAxis(ap=loc[:, t:t + 1], axis=0))
            c_ap = cb[:, 0:M]
            pairs = cb[:, M:W].rearrange("p (n c) -> p n c", c=2)

            cnt = big.tile([P, K], f32, tag="cnt")
            for k in range(K):
                nc.vector.tensor_scalar(
                    out=scrs[k % NSCR][:], in0=c_ap, scalar1=ut[:, t, k:k + 1],
                    scalar2=0.0, op0=LT, op1=ADD,
                    accum_out=cnt[:, k:k + 1])
            ix = big.tile([P, K], i16, tag="ix")
            nc.vector.tensor_copy(ix[:], cnt[:])

            g = big.tile([P, K * G, 2], f32, tag="g")
            nc.gpsimd.ap_gather(
                out_ap=g[:], in_ap=pairs, idxs_ap=ix[:],
                channels=P, num_elems=N, d=2, num_idxs=K * G)
            gs[t] = g
            # software pipelining: extract the PREVIOUS tile's gather while
            # Pool runs this tile's ap_gather, keeping DVE fed.
            if t >= 1:
                extract(t - 1, gs[t - 1])
        extract(T - 1, gs[T - 1])
    nc.compile()
    _cache["nc"] = nc
    return nc


def _prep(trg_seq, uniforms, knn_results, probs_table, cum_probs_table):
    """Host-side shard prep: per-core input dicts + combined table."""
    trg = np.asarray(trg_seq, dtype=np.int32)
    uni = np.ascontiguousarray(np.asarray(uniforms, dtype=np.float32))
    probs = np.asarray(probs_table, dtype=np.float32)
    cum = np.asarray(cum_probs_table, dtype=np.float32)
    knnf = np.asarray(knn_results).astype(np.float32)  # exact: ids < 2^24

    comb = np.empty((NLOC, W), dtype=np.float32)
    comb[:, 0:M] = cum[1:, 0:M]
    pv = comb[:, M:].reshape(NLOC, N, 2)
    pv[:, :, 0] = probs[1:]
    pv[:, :, 1] = knnf
    comb = np.ascontiguousarray(comb)

    mask = np.zeros((P, G), dtype=np.float32)
    mask[np.arange(P), np.arange(P) % G] = 1.0

    in_maps = []
    for c in range(NCORES):
        sl = slice(c * RPC, (c + 1) * RPC)
        locm1 = trg[sl, 1].reshape(T, P).T - 1  # [P, T], row l = t*128+p
        in_maps.append({
            "locm1": np.ascontiguousarray(locm1),
            "uni": np.ascontiguousarray(uni[sl].reshape(T, P, K).transpose(1, 0, 2)),
            "combt": comb,
            "mask": mask,
        })
    return in_maps


def kernel(trg_seq, k, user, uniforms, knn_results, probs_table, cum_probs_table,
           **_ignored):
    nc = _build()
    in_maps = _prep(trg_seq, uniforms, knn_results, probs_table, cum_probs_table)
    res = run_bass_kernel_spmd(nc, in_maps, core_ids=list(range(NCORES)))
    neg = np.empty((NCORES * RPC, K), dtype=np.int32)
    prob = np.empty((NCORES * RPC, K), dtype=np.float32)
    for c in range(NCORES):
        sl = slice(c * RPC, (c + 1) * RPC)
        o = res.results[c]["outv"].transpose(1, 0, 2, 3)  # [T,P,K,2] rows l=t*128+p
        prob[sl] = o[:, :, :, 0].reshape(RPC, K)
        neg[sl] = o[:, :, :, 1].reshape(RPC, K).astype(np.int32)
    return neg, prob
